# revision 1
# baseline (speedup 1.0000x reference)
"""Expert-parallel MoE FFN kernel for 8 Trainium2 NeuronCores.

Math (per expert e): out = gelu(x_e @ w1_e + b1_e) @ w2_e + b2_e
  x: [B=2, E=8, N=1024, D=1024], w1: [E, D, F=4096], b1: [E, F],
  w2: [E, F, D], b2: [E, D]  ->  out: [B, E, N, D]

Sharding: one expert per core (the e axis), outputs gathered on host.

Per-core kernel strategy (all matmuls in fp32r at full PE rate):
  - x_e [2048, 1024] is PE-transposed block-wise into xT [d, tok] in SBUF.
  - mm1: psum[f,tok] += w1[d,f].T @ xT[d,tok]  (f chunks of 128, tok 512)
  - ACT applies exact Gelu with per-partition bias b1[f] while moving
    PSUM -> SBUF hT [f, tok] (fp32r).
  - mm2: psum[tok,d] += hT[f,tok].T @ w2[f,d]  (accumulated over all f)
  - DVE adds broadcast b2 tile; result stores to DRAM in the natural
    [tok, d] layout (no output transpose needed).
Weights are streamed per 512-token block with large multi-dim DMAs
(dma_start issue rate, not HBM bandwidth, is the limiting resource).
Input loads ride the SP HWDGE ring; output stores ride the ACT ring so
next-block prefetch is never head-of-line blocked behind stores.
"""

import sys

for _p in ("/opt/trn_rl_repo", "/opt/pypackages"):
    if _p not in sys.path:
        sys.path.append(_p)

import numpy as np

B, E, N, D, F = 2, 8, 1024, 1024, 4096
TOK = B * N  # tokens per expert
TB = 512  # token block
NBLK = TOK // TB
nD = D // 128
nF = F // 128
nTS = TB // 128

_CACHE: dict = {}


def _build(reps: int = 1):
    import concourse.bacc as bacc
    import concourse.bass as bass
    import concourse.tile as tile
    from concourse import mybir
    from concourse.masks import make_identity

    F32 = mybir.dt.float32
    F32R = mybir.dt.float32r
    GELU = mybir.ActivationFunctionType.Gelu
    ADD = mybir.AluOpType.add

    nc = bacc.Bacc("TRN2", target_bir_lowering=False, debug=False, num_devices=8)

    x = nc.dram_tensor("x", [TOK, D], F32R, kind="ExternalInput").ap()
    w1 = nc.dram_tensor("w1", [D, F], F32R, kind="ExternalInput").ap()
    b1 = nc.dram_tensor("b1", [F], F32, kind="ExternalInput").ap()
    w2 = nc.dram_tensor("w2", [F, D], F32R, kind="ExternalInput").ap()
    b2 = nc.dram_tensor("b2", [D], F32, kind="ExternalInput").ap()
    out = nc.dram_tensor("out", [TOK, D], F32, kind="ExternalOutput").ap()

    # multi-dim views for coalesced DMAs
    x4 = x.rearrange("(blk q p) (dc c) -> blk q p dc c", q=nTS, p=128, c=128)
    w1_4 = w1.rearrange("(dc p) (fg f) -> dc p fg f", p=128, f=512)
    w2_4 = w2.rearrange("(fq fc p) (dh c) -> fq fc p dh c", fc=4, p=128, c=512)
    out4 = out.rearrange("(blk ts p) (dh c) -> blk ts p dh c", ts=nTS, p=128, c=512)

    with tile.TileContext(nc) as tc:
        with (
            tc.tile_pool(name="consts", bufs=1) as consts,
            tc.tile_pool(name="xTp", bufs=1) as xTp,
            tc.tile_pool(name="hTp", bufs=1) as hTp,
            tc.tile_pool(name="xlp", bufs=1) as xlp,
            tc.tile_pool(name="w1p", bufs=2) as w1p,
            tc.tile_pool(name="w2p", bufs=4) as w2p,
            tc.tile_pool(name="op", bufs=2) as op,
            tc.tile_pool(name="ps", bufs=8, space="PSUM") as ps,
        ):
            ident32 = consts.tile([128, 128], F32, tag="ident32")
            make_identity(nc, ident32)
            ident = consts.tile([128, 128], F32R, tag="ident")
            nc.gpsimd.tensor_copy(ident, ident32)

            b1_t = consts.tile([128, nF], F32, tag="b1")
            nc.sync.dma_start(out=b1_t, in_=b1.rearrange("(c p) -> p c", p=128))
            b2_t = consts.tile([128, D], F32, tag="b2")
            nc.gpsimd.dma_start(
                out=b2_t,
                in_=bass.AP(tensor=b2.tensor, offset=b2.offset, ap=[[0, 128], [1, D]]),
            )

            for blk in range(NBLK * reps):
                blk = blk % NBLK

                # --- load + transpose x block into xT [d, tok] (fp32r) ---
                xt = xlp.tile([128, nTS, nD, 128], F32R, tag="xl")
                nc.sync.dma_start(
                    out=xt, in_=x4[blk].rearrange("q p dc c -> p q dc c")
                )
                xT_t = xTp.tile([128, nD, TB], F32R, tag="xT")
                for dp in range(nD // 2):
                    # two PSUM tiles interleaved so PE overlaps weight loads
                    pta = ps.tile([128, TB], F32R, tag="ps", name=f"pta_{blk}_{dp}")
                    ptb = ps.tile([128, TB], F32R, tag="ps", name=f"ptb_{blk}_{dp}")
                    for q in range(nTS):
                        nc.tensor.transpose(
                            pta[:, q * 128 : (q + 1) * 128], xt[:, q, 2 * dp, :], ident
                        )
                        nc.tensor.transpose(
                            ptb[:, q * 128 : (q + 1) * 128], xt[:, q, 2 * dp + 1, :], ident
                        )
                    nc.vector.tensor_copy(xT_t[:, 2 * dp, :], pta)
                    nc.vector.tensor_copy(xT_t[:, 2 * dp + 1, :], ptb)

                # --- mm1 + gelu: hT [f, tok] (fp32r) ---
                hT_t = hTp.tile([128, nF, TB], F32R, tag="hT")
                for fg in range(nF // 4):  # f groups of 512
                    wt = w1p.tile([128, nD, 512], F32R, tag="w1")
                    nc.sync.dma_start(
                        out=wt, in_=w1_4[:, :, fg].rearrange("dc p f -> p dc f")
                    )
                    for fp in range(2):  # interleave two fc accumulations
                        fca = fg * 4 + 2 * fp
                        fcb = fca + 1
                        pha = ps.tile([128, TB], F32, tag="ps", name=f"pha_{blk}_{fg}_{fp}")
                        phb = ps.tile([128, TB], F32, tag="ps", name=f"phb_{blk}_{fg}_{fp}")
                        for dc in range(nD):
                            nc.tensor.matmul(
                                pha,
                                wt[:, dc, (2 * fp) * 128 : (2 * fp + 1) * 128],
                                xT_t[:, dc, :],
                                start=(dc == 0),
                                stop=(dc == nD - 1),
                            )
                            nc.tensor.matmul(
                                phb,
                                wt[:, dc, (2 * fp + 1) * 128 : (2 * fp + 2) * 128],
                                xT_t[:, dc, :],
                                start=(dc == 0),
                                stop=(dc == nD - 1),
                            )
                        nc.scalar.activation(
                            hT_t[:, fca, :], pha, GELU, bias=b1_t[:, fca : fca + 1],
                            scale=1.0,
                        )
                        nc.scalar.activation(
                            hT_t[:, fcb, :], phb, GELU, bias=b1_t[:, fcb : fcb + 1],
                            scale=1.0,
                        )

                # --- mm2 + b2: out [tok, d] ---
                for dh in range(D // 512):
                    pos = [
                        ps.tile([128, 512], F32, tag="ps", name=f"po_{blk}_{dh}_{i}")
                        for i in range(nTS)
                    ]
                    for fq in range(nF // 4):  # f chunks of 4x128
                        wt2 = w2p.tile([128, 4, 512], F32R, tag="w2")
                        nc.sync.dma_start(
                            out=wt2, in_=w2_4[fq, :, :, dh].rearrange("fc p c -> p fc c")
                        )
                        for fci in range(4):
                            fc = fq * 4 + fci
                            for ts in range(nTS):
                                nc.tensor.matmul(
                                    pos[ts],
                                    hT_t[:, fc, ts * 128 : (ts + 1) * 128],
                                    wt2[:, fci, :],
                                    start=(fc == 0),
                                    stop=(fc == nF - 1),
                                )
                    ot = op.tile([128, nTS, 512], F32, tag="o")
                    for ts in range(nTS):
                        nc.vector.tensor_tensor(
                            out=ot[:, ts, :],
                            in0=pos[ts],
                            in1=b2_t[:, dh * 512 : (dh + 1) * 512],
                            op=ADD,
                        )
                    nc.scalar.dma_start(
                        out=out4[blk, :, :, dh].rearrange("ts p c -> p ts c"), in_=ot
                    )

    nc.compile()
    return nc


def _get_nc(reps: int = 1):
    key = f"nc{reps}"
    if key not in _CACHE:
        _CACHE[key] = _build(reps)
    return _CACHE[key]


def kernel(x, w1, b1, w2, b2):
    from concourse.bass_utils import run_bass_kernel_spmd

    x = np.asarray(x, dtype=np.float32)
    w1 = np.asarray(w1, dtype=np.float32)
    b1 = np.asarray(b1, dtype=np.float32)
    w2 = np.asarray(w2, dtype=np.float32)
    b2 = np.asarray(b2, dtype=np.float32)

    nc = _get_nc()
    in_maps = []
    for e in range(E):
        in_maps.append(
            {
                "x": np.ascontiguousarray(x[:, e]).reshape(TOK, D),
                "w1": np.ascontiguousarray(w1[e]),
                "b1": np.ascontiguousarray(b1[e]),
                "w2": np.ascontiguousarray(w2[e]),
                "b2": np.ascontiguousarray(b2[e]),
            }
        )
    res = run_bass_kernel_spmd(nc, in_maps, list(range(E)))
    out = np.empty((B, E, N, D), np.float32)
    for e in range(E):
        out[:, e] = res.results[e]["out"].reshape(B, N, D)
    return out



# revision 2
# speedup vs baseline: 1.5909x; 1.5909x over previous
"""Expert-parallel MoE FFN kernel for 8 Trainium2 NeuronCores.

Math (per expert e): out = gelu(x_e @ w1_e + b1_e) @ w2_e + b2_e
  x: [B=2, E=8, N=1024, D=1024], w1: [E, D, F=4096], b1: [E, F],
  w2: [E, F, D], b2: [E, D]  ->  out: [B, E, N, D]

Sharding: one expert per core (the e axis of every tensor), outputs
gathered on host — the distributed path the original module implements
with all_gather + split_by_rank.

Per-core kernel (all matmuls bf16 into fp32 PSUM, measured at the PE
clock floor for this part — the moving operand streams 1 col/cycle and
weight loads are fully hidden, so 2048 N=512 matmuls is the hardware
minimum):
  - x arrives host-transposed as xT [d, tok] (bf16), so the device does
    no transposes at all. x and w2 are SBUF-resident the whole kernel;
    w1 streams per 512-wide f-group.
  - mm1: psum[f128, tok512] += w1[d, f].T @ xT[d, tok] over 8 k-tiles;
    ACT applies exact GELU with per-partition bias b1 while moving
    PSUM -> SBUF hT [f, tok] (bf16). Tokens processed in halves of 1024
    so hT fits SBUF alongside resident w2.
  - mm2: psum[tok128, d512] += hT[f, tok].T @ w2[f, d] over 32 f-tiles;
    DVE fuses the b2 broadcast add while moving PSUM -> SBUF fp32, then
    the result DMAs out in natural [tok, d] layout.
"""

import sys

for _p in ("/opt/trn_rl_repo", "/opt/pypackages"):
    if _p not in sys.path:
        sys.path.append(_p)

import numpy as np

B, E, N, D, F = 2, 8, 1024, 1024, 4096
TOK = B * N          # tokens per expert (2048)
HALF = TOK // 2      # token half (1024)
nD = D // 128        # 8 d-tiles
nF = F // 128        # 32 f-tiles
TB = 512             # matmul moving width

_CACHE: dict = {}


def _build(reps: int = 1):
    import concourse.bacc as bacc
    import concourse.bass as bass
    import concourse.tile as tile
    from concourse import mybir

    F32 = mybir.dt.float32
    BF16 = mybir.dt.bfloat16
    GELU = mybir.ActivationFunctionType.Gelu
    MULT = mybir.AluOpType.mult
    ADD = mybir.AluOpType.add

    nc = bacc.Bacc("TRN2", target_bir_lowering=False, debug=False, num_devices=8)

    xh = nc.dram_tensor("xh", [D, TOK], BF16, kind="ExternalInput").ap()
    w1h = nc.dram_tensor("w1h", [nD, 128, F], BF16, kind="ExternalInput").ap()
    w2h = nc.dram_tensor("w2h", [nF, 128, D], BF16, kind="ExternalInput").ap()
    b1 = nc.dram_tensor("b1", [F], F32, kind="ExternalInput").ap()
    b2 = nc.dram_tensor("b2", [D], F32, kind="ExternalInput").ap()
    out = nc.dram_tensor("out", [TOK, D], F32, kind="ExternalOutput").ap()

    x_v = xh.rearrange("(dt p) tok -> p dt tok", p=128)
    w1_vg = w1h.rearrange("dt p (fg f) -> p dt fg f", f=512)
    w2_v = w2h.rearrange("ft p d -> p ft d")
    out_v = out.rearrange("(hf tt p) (dh c) -> hf tt p dh c", hf=2, p=128, c=TB)

    with tile.TileContext(nc) as tc:
        with (
            tc.tile_pool(name="consts", bufs=1) as consts,
            tc.tile_pool(name="xp", bufs=1) as xp,
            tc.tile_pool(name="w2p", bufs=1) as w2p,
            tc.tile_pool(name="hp", bufs=1) as hp,
            tc.tile_pool(name="w1p", bufs=2) as w1p,
            tc.tile_pool(name="op", bufs=4) as op,
            tc.tile_pool(name="ps", bufs=8, space="PSUM") as ps,
        ):
            b1_t = consts.tile([128, nF], F32, tag="b1")
            nc.sync.dma_start(out=b1_t, in_=b1.rearrange("(fc p) -> p fc", p=128))
            b2_t = consts.tile([128, D], F32, tag="b2")
            nc.gpsimd.dma_start(
                out=b2_t,
                in_=bass.AP(tensor=b2.tensor, offset=b2.offset, ap=[[0, 128], [1, D]]),
            )

            # x resident as 4 chunk tiles so mm1 starts after the first 1MB
            xq_t = []
            for q in range(TOK // TB):
                xq = xp.tile([128, nD, TB], BF16, tag=f"xq{q}")
                nc.sync.dma_start(out=xq, in_=x_v[:, :, q * TB : (q + 1) * TB])
                xq_t.append(xq)
            w2h_t = w2p.tile([128, nF, D], BF16, tag="w2h")
            nc.sync.dma_start(out=w2h_t, in_=w2_v)

            for rep in range(reps):
                for hf in range(2):  # token halves
                    t0 = hf * HALF
                    # ---- mm1 + gelu -> hT [f, tok] (bf16) ----
                    hh_t = hp.tile(
                        [128, nF, HALF], BF16, tag="hh", name=f"hh_{rep}_{hf}"
                    )
                    for fg in range(nF // 4):  # f-groups of 512
                        w1h_g = w1p.tile(
                            [128, nD, 512], BF16, tag="w1h",
                            name=f"w1h_{rep}_{hf}_{fg}",
                        )
                        nc.sync.dma_start(out=w1h_g, in_=w1_vg[:, :, fg])
                        for fi in range(4):
                            fc = fg * 4 + fi
                            fs = fi * 128
                            for thc in range(HALF // TB):
                                xq = xq_t[(t0 + thc * TB) // TB]
                                pt = ps.tile(
                                    [128, TB], F32, tag="ps",
                                    name=f"p1_{rep}_{hf}_{fc}_{thc}",
                                )
                                for k in range(nD):
                                    nc.tensor.matmul(
                                        pt,
                                        w1h_g[:, k, fs : fs + 128],
                                        xq[:, k, :],
                                        start=(k == 0),
                                        stop=(k == nD - 1),
                                    )
                                nc.scalar.activation(
                                    hh_t[:, fc, thc * TB : thc * TB + TB], pt,
                                    GELU, bias=b1_t[:, fc : fc + 1], scale=1.0,
                                )

                    # ---- mm2 + b2 -> out [tok, d] (fp32) ----
                    for dh in range(D // TB):
                        ds = dh * TB
                        for tt in range(8):  # token tiles of 128 in this half
                            hts = tt * 128
                            pt = ps.tile(
                                [128, TB], F32, tag="ps",
                                name=f"p2_{rep}_{hf}_{dh}_{tt}",
                            )
                            for k in range(nF):
                                nc.tensor.matmul(
                                    pt,
                                    hh_t[:, k, hts : hts + 128],
                                    w2h_t[:, k, ds : ds + TB],
                                    start=(k == 0),
                                    stop=(k == nF - 1),
                                )
                            ot = op.tile(
                                [128, TB], F32, tag="o",
                                name=f"o_{rep}_{hf}_{dh}_{tt}",
                            )
                            nc.vector.scalar_tensor_tensor(
                                out=ot,
                                in0=pt,
                                scalar=1.0,
                                in1=b2_t[:, ds : ds + TB],
                                op0=MULT,
                                op1=ADD,
                            )
                            nc.scalar.dma_start(out=out_v[hf, tt, :, dh], in_=ot)

    nc.compile()
    return nc


def _get_nc(reps: int = 1):
    key = f"nc{reps}"
    if key not in _CACHE:
        _CACHE[key] = _build(reps)
    return _CACHE[key]


def _prep_core_inputs(x, w1, b1, w2, b2, e):
    from concourse import mybir

    BFnp = mybir.dt.np(mybir.dt.bfloat16)
    xT = np.ascontiguousarray(x[:, e].reshape(TOK, D).T)  # [D, TOK]
    return {
        "xh": xT.astype(BFnp),
        "w1h": np.ascontiguousarray(w1[e].astype(BFnp).reshape(nD, 128, F)),
        "w2h": np.ascontiguousarray(w2[e].astype(BFnp).reshape(nF, 128, D)),
        "b1": np.ascontiguousarray(b1[e]),
        "b2": np.ascontiguousarray(b2[e]),
    }


def kernel(x, w1, b1, w2, b2):
    from concourse.bass_utils import run_bass_kernel_spmd

    x = np.asarray(x, dtype=np.float32)
    w1 = np.asarray(w1, dtype=np.float32)
    b1 = np.asarray(b1, dtype=np.float32)
    w2 = np.asarray(w2, dtype=np.float32)
    b2 = np.asarray(b2, dtype=np.float32)

    nc = _get_nc()
    in_maps = [_prep_core_inputs(x, w1, b1, w2, b2, e) for e in range(E)]
    res = run_bass_kernel_spmd(nc, in_maps, list(range(E)))
    out = np.empty((B, E, N, D), np.float32)
    for e in range(E):
        out[:, e] = res.results[e]["out"].reshape(B, N, D)
    return out
